# revision 12
# baseline (speedup 1.0000x reference)
"""Trainium2 Bass kernel for nn_LowRankLinear (y = x @ (U@V).T + bias).

Strategy:
  - Data-parallel: shard the 8192 tokens across 8 NeuronCores (1024 each).
  - Low-rank factorization on-device: t.T = (V @ x.T)  [rank x tok], then
    y.T = U @ t + bias — 34 GFLOP total instead of 283 GFLOP for the
    materialized-W reference.
  - All operands in bf16 (inputs quantized on host, output upcast on host):
    halves all DMA bytes vs fp32 (21 MB/core vs 42 MB) while the PE runs
    bf16 at the same 1 column/cycle as the fp32r path. rel-err ~3.5e-3,
    well within the 2e-2 gate. PSUM accumulation stays fp32.
  - With bf16 the whole x shard (64 KB/partition) is SBUF-resident: no
    tile rotation, no WAR hazards.
  - Critical path is PE busy (256 MMs × ~220 ns ≈ 57 us) + first-data
    latency + drain. So: tiny first transfers (V chunk 0 = 64 KB, x chunk
    0 = 256 KB) dispatched from the scalar/vector sequencers (free at
    ~5.8 us, vs sync's ~7.2 us preamble), a few warm-up matmuls on zeroed
    tiles so the HAM clock-gate reaches full rate before real data lands,
    then a single continuous sync-queue stream sized so DMA stays ahead
    of the PE's ~350 GB/s consumption.
  - y.T orientation makes bias per-PARTITION; PSUM eviction alternates
    DVE (tensor_scalar_add) and ACT (activation Identity+bias).

Self-contained: hardcodes shapes from the problem spec; only needs the
concourse repo at /opt/trn_rl_repo (container-provided).
"""

import sys

if "/opt/trn_rl_repo" not in sys.path:
    sys.path.insert(0, "/opt/trn_rl_repo")

import numpy as np

import concourse.mybir as mybir
import concourse.tile as tile
from concourse import bacc
from concourse.bass_utils import run_bass_kernel_spmd

# Problem shapes (hardcoded per contract)
TOKENS = 8192
IN_F = 4096
OUT_F = 4096
RANK = 256
N_CORES = 8
TPC = TOKENS // N_CORES  # tokens per core = 1024

P = 128  # partitions
NG = 512  # moving free-dim per matmul (PSUM bank limit for f32)
KC = IN_F // P  # 32 k-chunks for matmul1
RC = RANK // P  # 2 rank chunks
G = TPC // NG  # 2 halves of the token range
OFT = OUT_F // P  # 32 out_f tiles for matmul2

# x DMA granularity in k-chunks: tiny first chunks so matmul1 starts at
# ~7.5 us, then 1 MB blocks that stream faster than the PE consumes.
X_GROUPS = [1, 1, 2, 4, 4, 4, 4, 4, 4, 4]
# V DMA granularity in k-chunks (V chunk = 64 KB)
V_GROUPS = [1, 3, 12, 16]

NWARM = 4  # HAM warm-up matmuls on zeroed tiles before real data lands

F32 = mybir.dt.float32
MMDT = mybir.dt.bfloat16  # halves all DMA bytes; PE still 1 col/cycle
ODT = mybir.dt.bfloat16  # output stored bf16, upcast to f32 on host

_CACHE = {}


def _build(mmdt):
    nc = bacc.Bacc(
        trn_type="TRN2", target_bir_lowering=False, debug=False, num_devices=N_CORES
    )
    # All inputs pre-packed on host into the exact SBUF images so every DMA
    # is a flat 2D copy with contiguous per-partition lines.
    xP = nc.dram_tensor("xP", [P, KC * TPC], mmdt, kind="ExternalInput")
    vP = nc.dram_tensor("vP", [P, KC * RANK], mmdt, kind="ExternalInput")
    uP = nc.dram_tensor("uP", [P, RC * OUT_F], mmdt, kind="ExternalInput")
    # bias in column layout: bias_col[p, of] = bias[of*128 + p]
    biasc = nc.dram_tensor("biasc", [P, OFT], F32, kind="ExternalInput")
    yT = nc.dram_tensor("yT", [OUT_F, TPC], ODT, kind="ExternalOutput")

    with tile.TileContext(nc) as tc:
        with (
            tc.tile_pool(name="const", bufs=1) as cp,
            tc.tile_pool(name="yp", bufs=6) as yp,
            tc.tile_pool(name="pt", bufs=4, space="PSUM") as ptp,
            tc.tile_pool(name="py", bufs=4, space="PSUM") as pyp,
        ):
            # ---- resident tensors ----
            # Tile deps are per-TILE (not per-range): every tile below is
            # written by exactly ONE dma_start so consumers wait only for
            # the bytes they actually read.
            v_tiles = []  # (cstart, cn, tile): V.T chunk groups
            c0 = 0
            for i, cn in enumerate(V_GROUPS):
                v_tiles.append((c0, cn, cp.tile([P, cn * RANK], mmdt, name=f"vt{i}")))
                c0 += cn
            # U.T quarters: u_tiles[r][half] covers rank-tile r, of half
            u_tiles = [
                [cp.tile([P, OUT_F // 2], mmdt, name=f"ut{r}{h}") for h in range(2)]
                for r in range(RC)
            ]
            # t.T per (r, g): written by one eviction op each
            t_tiles = [
                [cp.tile([P, NG], mmdt, name=f"tt{r}{g}") for g in range(G)]
                for r in range(RC)
            ]
            bcol = cp.tile([P, OFT], F32)  # per-partition bias columns
            wmv = cp.tile([P, NG], mmdt)  # warm-up moving operand (zeros)
            wwt = cp.tile([P, P], mmdt)  # warm-up weights (zeros)
            # whole x shard resident: one tile per DMA group
            x_tiles = []
            c0 = 0
            for i, gsz in enumerate(X_GROUPS):
                x_tiles.append((c0, gsz, cp.tile([P, gsz * TPC], mmdt, name=f"xt{i}")))
                c0 += gsz

            def v_slice(c):
                # (tile, column offset) for V.T chunk c
                for cstart, cn, vt in v_tiles:
                    if cstart <= c < cstart + cn:
                        return vt, (c - cstart) * RANK
                raise AssertionError(c)

            # ---- PE warm-up: zeroed tiles, no DMA dependency ----
            nc.vector.memset(wmv[:], 0.0)
            nc.vector.memset(wwt[:], 0.0)
            wpt = pyp.tile([P, NG], F32, name="warm", tag="py")
            for _ in range(NWARM):
                nc.tensor.matmul(wpt[:], wwt[:], wmv[:], start=True, stop=True)

            def load_v(eng, i):
                cstart, cn, vt = v_tiles[i]
                eng.dma_start(
                    vt[:], vP[:, cstart * RANK : (cstart + cn) * RANK]
                )

            def load_u(eng, r, half):
                sl = slice(
                    r * OUT_F + half * (OUT_F // 2),
                    r * OUT_F + (half + 1) * (OUT_F // 2),
                )
                eng.dma_start(u_tiles[r][half][:], uP[:, sl])

            def load_x(eng, i):
                cstart, gsz, xt = x_tiles[i]
                eng.dma_start(
                    xt[:], xP[:, cstart * TPC : (cstart + gsz) * TPC]
                )

            # ---- inflow ----
            # The scalar sequencer finishes its preamble ~1.4 us before sync:
            # it carries the launch-critical transfers (V c0 + x c0/c1).
            load_v(nc.scalar, 0)  # V c0 (64 KB)
            load_x(nc.scalar, 0)  # x c0 (256 KB)
            load_x(nc.scalar, 1)  # x c1
            load_v(nc.scalar, 1)  # V c1-3
            nc.scalar.dma_start(bcol[:], biasc[:])
            # sync stream: stays ahead of PE consumption (~350 GB/s)
            load_x(nc.sync, 2)  # x c2-3
            load_v(nc.sync, 2)  # V c4-15
            load_x(nc.sync, 3)  # x c4-7
            load_x(nc.sync, 4)  # x c8-11
            load_v(nc.sync, 3)  # V c16-31
            load_x(nc.sync, 5)  # x c12-15
            load_x(nc.sync, 6)  # x c16-19
            load_x(nc.sync, 7)  # x c20-23
            load_u(nc.sync, 0, 0)  # U for of 0-15 ...
            load_u(nc.sync, 1, 0)  # ... lands ~27 us, needed ~38 us
            load_x(nc.sync, 8)  # x c24-27
            load_x(nc.sync, 9)  # x c28-31
            load_u(nc.sync, 0, 1)  # U for of 16-31
            load_u(nc.sync, 1, 1)

            # ---- matmul1: t.T = sum_c V.T_c.T @ x.T_c over both token halves ----
            pt = [
                ptp.tile([P, NG], F32, name=f"pt{r}_{g}", tag="pt")
                for r in range(RC)
                for g in range(G)
            ]
            for i, (cstart, gsz, xt) in enumerate(x_tiles):
                for cc in range(gsz):
                    c = cstart + cc
                    order = (
                        [(r, g) for g in range(G) for r in range(RC)]
                        if c == KC - 1
                        else [(r, g) for r in range(RC) for g in range(G)]
                    )
                    vt, voff = v_slice(c)
                    for r, g in order:
                        nc.tensor.matmul(
                            pt[r * G + g][:],
                            vt[:, voff + r * P : voff + (r + 1) * P],
                            xt[:, cc * TPC + g * NG : cc * TPC + (g + 1) * NG],
                            start=(c == 0),
                            stop=(c == KC - 1),
                        )
            # f32 PSUM -> bf16 SBUF rounding copies; g-major so matmul2's g0
            # operands are ready first; r0 on DVE, r1 on ACT in parallel.
            for g in range(G):
                for r in range(RC):
                    if r == 0:
                        nc.vector.tensor_copy(t_tiles[r][g][:], pt[r * G + g][:])
                    else:
                        nc.scalar.copy(t_tiles[r][g][:], pt[r * G + g][:])

            # ---- matmul2: y.T[of] = U.T_of.T @ t.T + bias ----
            # Eviction alternates DVE / ACT so both engines share the load.
            for of in range(OFT):
                ysb = yp.tile([P, TPC], ODT)
                for g in range(G):
                    pyt = pyp.tile([P, NG], F32, tag="py")
                    for r in range(RC):
                        half = of // (OFT // 2)
                        lof = of - half * (OFT // 2)
                        nc.tensor.matmul(
                            pyt[:],
                            u_tiles[r][half][:, lof * P : (lof + 1) * P],
                            t_tiles[r][g][:],
                            start=(r == 0),
                            stop=(r == RC - 1),
                        )
                    if g == 0:
                        nc.vector.tensor_scalar_add(
                            ysb[:, g * NG : (g + 1) * NG],
                            pyt[:],
                            bcol[:, of : of + 1],
                        )
                    else:
                        nc.scalar.activation(
                            ysb[:, g * NG : (g + 1) * NG],
                            pyt[:],
                            mybir.ActivationFunctionType.Identity,
                            bias=bcol[:, of : of + 1],
                        )
                nc.sync.dma_start(yT[of * P : (of + 1) * P, :], ysb[:])
    nc.compile()
    return nc


def _get_nc():
    key = MMDT
    if key not in _CACHE:
        _CACHE[key] = _build(key)
    return _CACHE[key]


def _prep_in_maps(x, U, V, bias):
    import ml_dtypes

    bf16 = ml_dtypes.bfloat16
    # Cast to bf16 first so the pack-transposes move half the bytes.
    x = np.asarray(x, dtype=np.float32).astype(bf16)
    V = np.asarray(V, dtype=np.float32).astype(bf16)
    U = np.asarray(U, dtype=np.float32).astype(bf16)
    # SBUF images: vsb[p, c*RANK+m] = V[m, c*128+p]; usb[p, r*OUT_F+o] = U[o, r*128+p]
    vp = np.ascontiguousarray(
        V.reshape(RANK, KC, P).transpose(2, 1, 0).reshape(P, KC * RANK)
    )
    up = np.ascontiguousarray(
        U.reshape(OUT_F, RC, P).transpose(2, 1, 0).reshape(P, RC * OUT_F)
    )
    bc = np.ascontiguousarray(np.asarray(bias, dtype=np.float32).reshape(OFT, P).T)
    in_maps = []
    for i in range(N_CORES):
        xs = x[i * TPC : (i + 1) * TPC, :]
        # xP[p, c*TPC+n] = x[n, c*128+p]
        xp_img = np.ascontiguousarray(
            xs.reshape(TPC, KC, P).transpose(2, 1, 0).reshape(P, KC * TPC)
        )
        in_maps.append({"xP": xp_img, "vP": vp, "uP": up, "biasc": bc})
    return in_maps


def _gather(res):
    # res.results[i]["yT"] is [OUT_F, TPC] bf16; full y is the token-major
    # concat of the transposes, upcast to f32 on host.
    yt = np.concatenate([res.results[i]["yT"] for i in range(N_CORES)], axis=1)
    return np.ascontiguousarray(yt.astype(np.float32).T)


def kernel(x, U, V, bias):
    nc = _get_nc()
    in_maps = _prep_in_maps(x, U, V, bias)
    res = run_bass_kernel_spmd(nc, in_maps, core_ids=list(range(N_CORES)))
    return _gather(res)


def run_profiled(x, U, V, bias, **trace_kwargs):
    """Like kernel() but with NTFF tracing; returns (y, BassKernelResults)."""
    nc = _get_nc()
    in_maps = _prep_in_maps(x, U, V, bias)
    res = run_bass_kernel_spmd(
        nc, in_maps, core_ids=list(range(N_CORES)), trace=True, **trace_kwargs
    )
    return _gather(res), res


# revision 13
# speedup vs baseline: 1.2346x; 1.2346x over previous
"""Trainium2 Bass kernel for nn_LowRankLinear (y = x @ (U@V).T + bias).

Strategy:
  - Data-parallel: shard the 8192 tokens across 8 NeuronCores (1024 each).
  - Low-rank factorization on-device: t.T = (V @ x.T)  [rank x tok], then
    y.T = U @ t + bias — 34 GFLOP total instead of 283 GFLOP for the
    materialized-W reference.
  - All operands in bf16 (inputs quantized on host, output upcast on host):
    halves all DMA bytes vs fp32 (21 MB/core vs 42 MB) while the PE runs
    bf16 at the same 1 column/cycle as the fp32r path. rel-err ~3.5e-3,
    well within the 2e-2 gate. PSUM accumulation stays fp32.
  - With bf16 the whole x shard (64 KB/partition) is SBUF-resident: no
    tile rotation, no WAR hazards.
  - Critical path is PE busy (256 MMs × ~220 ns ≈ 57 us) + first-data
    latency + drain. So: tiny first transfers (V chunk 0 = 64 KB, x chunk
    0 = 256 KB) dispatched from the scalar/vector sequencers (free at
    ~5.8 us, vs sync's ~7.2 us preamble), a few warm-up matmuls on zeroed
    tiles so the HAM clock-gate reaches full rate before real data lands,
    then a single continuous sync-queue stream sized so DMA stays ahead
    of the PE's ~350 GB/s consumption.
  - y.T orientation makes bias per-PARTITION; PSUM eviction alternates
    DVE (tensor_scalar_add) and ACT (activation Identity+bias).

Self-contained: hardcodes shapes from the problem spec; only needs the
concourse repo at /opt/trn_rl_repo (container-provided).
"""

import sys

if "/opt/trn_rl_repo" not in sys.path:
    sys.path.insert(0, "/opt/trn_rl_repo")

import numpy as np

import concourse.mybir as mybir
import concourse.tile as tile
from concourse import bacc
from concourse.bass_utils import run_bass_kernel_spmd

# Problem shapes (hardcoded per contract)
TOKENS = 8192
IN_F = 4096
OUT_F = 4096
RANK = 256
N_CORES = 8
TPC = TOKENS // N_CORES  # tokens per core = 1024

P = 128  # partitions
NG = 512  # moving free-dim per matmul (PSUM bank limit for f32)
KC = IN_F // P  # 32 k-chunks for matmul1
RC = RANK // P  # 2 rank chunks
G = TPC // NG  # 2 halves of the token range
OFT = OUT_F // P  # 32 out_f tiles for matmul2

# x DMA granularity in k-chunks: tiny first chunks so matmul1 starts at
# ~7.5 us, then 1 MB blocks that stream faster than the PE consumes.
X_GROUPS = [1, 1, 2, 4, 4, 4, 4, 4, 4, 4]
# V DMA granularity in k-chunks (V chunk = 64 KB)
V_GROUPS = [1, 3, 12, 16]

NWARM = 4  # HAM warm-up matmuls on zeroed tiles before real data lands

F32 = mybir.dt.float32
MMDT = mybir.dt.bfloat16  # halves all DMA bytes; PE still 1 col/cycle
ODT = mybir.dt.bfloat16  # output stored bf16, upcast to f32 on host

_CACHE = {}


def _build(mmdt):
    nc = bacc.Bacc(
        trn_type="TRN2", target_bir_lowering=False, debug=False, num_devices=N_CORES
    )
    # All inputs pre-packed on host into the exact SBUF images so every DMA
    # is a flat 2D copy with contiguous per-partition lines.
    xP = nc.dram_tensor("xP", [P, KC * TPC], mmdt, kind="ExternalInput")
    vP = nc.dram_tensor("vP", [P, KC * RANK], mmdt, kind="ExternalInput")
    uP = nc.dram_tensor("uP", [P, RC * OUT_F], mmdt, kind="ExternalInput")
    # bias in column layout: bias_col[p, of] = bias[of*128 + p]
    biasc = nc.dram_tensor("biasc", [P, OFT], F32, kind="ExternalInput")
    yT = nc.dram_tensor("yT", [OUT_F, TPC], ODT, kind="ExternalOutput")

    with tile.TileContext(nc) as tc:
        with (
            tc.tile_pool(name="const", bufs=1) as cp,
            tc.tile_pool(name="yp", bufs=6) as yp,
            tc.tile_pool(name="pt", bufs=4, space="PSUM") as ptp,
            tc.tile_pool(name="py", bufs=4, space="PSUM") as pyp,
        ):
            # ---- resident tensors ----
            # Tile deps are per-TILE (not per-range): every tile below is
            # written by exactly ONE dma_start so consumers wait only for
            # the bytes they actually read.
            v_tiles = []  # (cstart, cn, tile): V.T chunk groups
            c0 = 0
            for i, cn in enumerate(V_GROUPS):
                v_tiles.append((c0, cn, cp.tile([P, cn * RANK], mmdt, name=f"vt{i}")))
                c0 += cn
            # U.T quarters: u_tiles[r][half] covers rank-tile r, of half
            u_tiles = [
                [cp.tile([P, OUT_F // 2], mmdt, name=f"ut{r}{h}") for h in range(2)]
                for r in range(RC)
            ]
            # t.T per (r, g): written by one eviction op each
            t_tiles = [
                [cp.tile([P, NG], mmdt, name=f"tt{r}{g}") for g in range(G)]
                for r in range(RC)
            ]
            bcol = cp.tile([P, OFT], F32)  # per-partition bias columns
            wmv = cp.tile([P, NG], mmdt)  # warm-up moving operand (zeros)
            wwt = cp.tile([P, P], mmdt)  # warm-up weights (zeros)
            # whole x shard resident: one tile per DMA group
            x_tiles = []
            c0 = 0
            for i, gsz in enumerate(X_GROUPS):
                x_tiles.append((c0, gsz, cp.tile([P, gsz * TPC], mmdt, name=f"xt{i}")))
                c0 += gsz

            def v_slice(c):
                # (tile, column offset) for V.T chunk c
                for cstart, cn, vt in v_tiles:
                    if cstart <= c < cstart + cn:
                        return vt, (c - cstart) * RANK
                raise AssertionError(c)

            # ---- PE warm-up: zeroed tiles, no DMA dependency ----
            nc.vector.memset(wmv[:], 0.0)
            nc.vector.memset(wwt[:], 0.0)
            wpt = pyp.tile([P, NG], F32, name="warm", tag="py")
            for _ in range(NWARM):
                nc.tensor.matmul(wpt[:], wwt[:], wmv[:], start=True, stop=True)

            def load_v(eng, i):
                cstart, cn, vt = v_tiles[i]
                eng.dma_start(
                    vt[:], vP[:, cstart * RANK : (cstart + cn) * RANK]
                )

            def load_u(eng, r, half):
                sl = slice(
                    r * OUT_F + half * (OUT_F // 2),
                    r * OUT_F + (half + 1) * (OUT_F // 2),
                )
                eng.dma_start(u_tiles[r][half][:], uP[:, sl])

            def load_x(eng, i):
                cstart, gsz, xt = x_tiles[i]
                eng.dma_start(
                    xt[:], xP[:, cstart * TPC : (cstart + gsz) * TPC]
                )

            # ---- inflow ----
            # ALL bulk transfers go on the sync ring in FIFO priority order:
            # the SDMA engines starve the Act ring when the sync ring has fat
            # packets queued, so a second ring only helps for the tiny bias.
            load_v(nc.sync, 0)  # V c0 (64 KB) — first matmul gate
            load_x(nc.sync, 0)  # x c0 (256 KB)
            load_x(nc.sync, 1)  # x c1
            load_v(nc.sync, 1)  # V c1-3
            nc.scalar.dma_start(bcol[:], biasc[:])  # tiny, needed at ~39 us
            load_x(nc.sync, 2)  # x c2-3
            load_v(nc.sync, 2)  # V c4-15
            load_x(nc.sync, 3)  # x c4-7
            load_x(nc.sync, 4)  # x c8-11
            load_v(nc.sync, 3)  # V c16-31
            load_x(nc.sync, 5)  # x c12-15
            load_x(nc.sync, 6)  # x c16-19
            load_x(nc.sync, 7)  # x c20-23
            load_u(nc.sync, 0, 0)  # U for of 0-15 ...
            load_u(nc.sync, 1, 0)  # ... lands ~27 us, needed ~38 us
            load_x(nc.sync, 8)  # x c24-27
            load_x(nc.sync, 9)  # x c28-31
            load_u(nc.sync, 0, 1)  # U for of 16-31
            load_u(nc.sync, 1, 1)

            # ---- matmul1: t.T = sum_c V.T_c.T @ x.T_c over both token halves ----
            pt = [
                ptp.tile([P, NG], F32, name=f"pt{r}_{g}", tag="pt")
                for r in range(RC)
                for g in range(G)
            ]
            for i, (cstart, gsz, xt) in enumerate(x_tiles):
                for cc in range(gsz):
                    c = cstart + cc
                    order = (
                        [(r, g) for g in range(G) for r in range(RC)]
                        if c == KC - 1
                        else [(r, g) for r in range(RC) for g in range(G)]
                    )
                    vt, voff = v_slice(c)
                    for r, g in order:
                        nc.tensor.matmul(
                            pt[r * G + g][:],
                            vt[:, voff + r * P : voff + (r + 1) * P],
                            xt[:, cc * TPC + g * NG : cc * TPC + (g + 1) * NG],
                            start=(c == 0),
                            stop=(c == KC - 1),
                        )
            # f32 PSUM -> bf16 SBUF rounding copies; g-major so matmul2's g0
            # operands are ready first; r0 on DVE, r1 on ACT in parallel.
            for g in range(G):
                for r in range(RC):
                    if r == 0:
                        nc.vector.tensor_copy(t_tiles[r][g][:], pt[r * G + g][:])
                    else:
                        nc.scalar.copy(t_tiles[r][g][:], pt[r * G + g][:])

            # ---- matmul2: y.T[of] = U.T_of.T @ t.T + bias ----
            # Eviction alternates DVE / ACT so both engines share the load.
            for of in range(OFT):
                ysb = yp.tile([P, TPC], ODT)
                for g in range(G):
                    pyt = pyp.tile([P, NG], F32, tag="py")
                    for r in range(RC):
                        half = of // (OFT // 2)
                        lof = of - half * (OFT // 2)
                        nc.tensor.matmul(
                            pyt[:],
                            u_tiles[r][half][:, lof * P : (lof + 1) * P],
                            t_tiles[r][g][:],
                            start=(r == 0),
                            stop=(r == RC - 1),
                        )
                    if g == 0:
                        nc.vector.tensor_scalar_add(
                            ysb[:, g * NG : (g + 1) * NG],
                            pyt[:],
                            bcol[:, of : of + 1],
                        )
                    else:
                        nc.scalar.activation(
                            ysb[:, g * NG : (g + 1) * NG],
                            pyt[:],
                            mybir.ActivationFunctionType.Identity,
                            bias=bcol[:, of : of + 1],
                        )
                nc.sync.dma_start(yT[of * P : (of + 1) * P, :], ysb[:])
    nc.compile()
    return nc


def _get_nc():
    key = MMDT
    if key not in _CACHE:
        _CACHE[key] = _build(key)
    return _CACHE[key]


def _prep_in_maps(x, U, V, bias):
    import ml_dtypes

    bf16 = ml_dtypes.bfloat16
    # Cast to bf16 first so the pack-transposes move half the bytes.
    x = np.asarray(x, dtype=np.float32).astype(bf16)
    V = np.asarray(V, dtype=np.float32).astype(bf16)
    U = np.asarray(U, dtype=np.float32).astype(bf16)
    # SBUF images: vsb[p, c*RANK+m] = V[m, c*128+p]; usb[p, r*OUT_F+o] = U[o, r*128+p]
    vp = np.ascontiguousarray(
        V.reshape(RANK, KC, P).transpose(2, 1, 0).reshape(P, KC * RANK)
    )
    up = np.ascontiguousarray(
        U.reshape(OUT_F, RC, P).transpose(2, 1, 0).reshape(P, RC * OUT_F)
    )
    bc = np.ascontiguousarray(np.asarray(bias, dtype=np.float32).reshape(OFT, P).T)
    in_maps = []
    for i in range(N_CORES):
        xs = x[i * TPC : (i + 1) * TPC, :]
        # xP[p, c*TPC+n] = x[n, c*128+p]
        xp_img = np.ascontiguousarray(
            xs.reshape(TPC, KC, P).transpose(2, 1, 0).reshape(P, KC * TPC)
        )
        in_maps.append({"xP": xp_img, "vP": vp, "uP": up, "biasc": bc})
    return in_maps


def _gather(res):
    # res.results[i]["yT"] is [OUT_F, TPC] bf16; full y is the token-major
    # concat of the transposes, upcast to f32 on host.
    yt = np.concatenate([res.results[i]["yT"] for i in range(N_CORES)], axis=1)
    return np.ascontiguousarray(yt.astype(np.float32).T)


def kernel(x, U, V, bias):
    nc = _get_nc()
    in_maps = _prep_in_maps(x, U, V, bias)
    res = run_bass_kernel_spmd(nc, in_maps, core_ids=list(range(N_CORES)))
    return _gather(res)


def run_profiled(x, U, V, bias, **trace_kwargs):
    """Like kernel() but with NTFF tracing; returns (y, BassKernelResults)."""
    nc = _get_nc()
    in_maps = _prep_in_maps(x, U, V, bias)
    res = run_bass_kernel_spmd(
        nc, in_maps, core_ids=list(range(N_CORES)), trace=True, **trace_kwargs
    )
    return _gather(res), res
